# revision 1
# baseline (speedup 1.0000x reference)
"""KSparseLinear forward on 8 Trainium2 NeuronCores.

out = (x * mask) @ weight.T + bias, where mask keeps the top-k=64
|feature_importance| columns of the 4096 input features.

Strategy (data-parallel over the 65536-row batch, 8192 rows/core):
  - Host: compute top-k indices, gather the 64 relevant weight columns
    (wT_sel [64, 8]), broadcast bias to a [128, 8] tile.
  - Device, per 128-row tile of x:
      1. DMA the tile [128, 4096] in natural layout (full-bandwidth
         contiguous 16 KB/partition reads).  This is the roofline term:
         1 GiB total / 8 cores at ~358 GB/s/core.
      2. Gather the 64 selected columns with 32 paired strided-AP DVE
         copies -> x_sel [128, 64].
      3. PE transpose x_sel -> PSUM [64, 128]; ACT copy to SBUF.
      4. One PE matmul: out[128, 8] = x_selT.T @ wT_sel.
      5. DVE add bias (PSUM + bias -> SBUF), batched output DMA at end.
"""

import numpy as np

N_FULL, IN_F, OUT_F = 65536, 4096, 8
NCORES = 8
ROWS = N_FULL // NCORES  # 8192 rows per core
P = 128                  # partitions / rows per tile
NTILES = ROWS // P       # 64 tiles per core


def _make_pairs(idx_sorted):
    """Group sorted column indices into (offset, step, count) strided runs
    expressible as a single AP dim (count<=2 always works; equal-gap runs
    could be longer but pairs are guaranteed)."""
    pairs = []
    i = 0
    n = len(idx_sorted)
    while i < n:
        if i + 1 < n:
            a, b = int(idx_sorted[i]), int(idx_sorted[i + 1])
            pairs.append((a, b - a, 2))
            i += 2
        else:
            pairs.append((int(idx_sorted[i]), 1, 1))
            i += 1
    return pairs


def build_nc(pairs, k_sel):
    import concourse.bass as bass
    import concourse.mybir as mybir
    from concourse import tile
    from concourse.bacc import Bacc

    f32 = mybir.dt.float32
    nc = Bacc()

    x_d = nc.declare_dram_parameter("x", [ROWS, IN_F], f32, isOutput=False)
    wt_d = nc.declare_dram_parameter("wt", [k_sel, OUT_F], f32, isOutput=False)
    bias_d = nc.declare_dram_parameter("biasb", [P, OUT_F], f32, isOutput=False)
    id_d = nc.declare_dram_parameter("ident", [P, P], f32, isOutput=False)
    out_d = nc.declare_dram_parameter("out", [ROWS, OUT_F], f32, isOutput=True)

    with tile.TileContext(nc) as tc:
        with (
            tc.tile_pool(name="consts", bufs=1) as consts,
            tc.tile_pool(name="xin", bufs=6) as xin,
            tc.tile_pool(name="sel", bufs=3) as selp,
            tc.tile_pool(name="xt", bufs=3) as xtp,
            tc.tile_pool(name="psT", bufs=2, space=bass.MemorySpace.PSUM) as psT,
            tc.tile_pool(name="psO", bufs=2, space=bass.MemorySpace.PSUM) as psO,
            tc.tile_pool(name="outp", bufs=1) as outp,
        ):
            wt_sb = consts.tile([k_sel, OUT_F], f32)
            nc.sync.dma_start(out=wt_sb[:], in_=wt_d[:, :])
            bias_sb = consts.tile([P, OUT_F], f32)
            nc.sync.dma_start(out=bias_sb[:], in_=bias_d[:, :])
            id_sb = consts.tile([P, P], f32)
            nc.sync.dma_start(out=id_sb[:], in_=id_d[:, :])

            out_all = outp.tile([P, NTILES * OUT_F], f32)

            for t in range(NTILES):
                x_tile = xin.tile([P, IN_F], f32)
                # alternate the two HWDGE rings (SP + ACT) so two loads
                # stream concurrently; one ring alone under-delivers
                eng = nc.sync if t % 2 == 0 else nc.scalar
                eng.dma_start(out=x_tile[:], in_=x_d[t * P:(t + 1) * P, :])

                xsel = selp.tile([P, k_sel], f32)
                col = 0
                for (off, step, cnt) in pairs:
                    if cnt == 1:
                        src = x_tile[:, off:off + 1]
                    else:
                        src = x_tile[:, off:off + step * (cnt - 1) + 1:step]
                    nc.vector.tensor_copy(xsel[:, col:col + cnt], src)
                    col += cnt

                xT_ps = psT.tile([k_sel, P], f32)
                nc.tensor.transpose(xT_ps[:], xsel[:], id_sb[:])
                xT_sb = xtp.tile([k_sel, P], f32)
                nc.scalar.copy(xT_sb[:], xT_ps[:])

                o_ps = psO.tile([P, OUT_F], f32)
                nc.tensor.matmul(o_ps[:], xT_sb[:], wt_sb[:], start=True, stop=True)
                nc.vector.tensor_add(
                    out_all[:, t * OUT_F:(t + 1) * OUT_F], o_ps[:], bias_sb[:]
                )

            out_view = out_d[:, :].rearrange("(t p) o -> p t o", p=P)
            out_all_3d = out_all[:].rearrange("p (t o) -> p t o", o=OUT_F)
            nc.sync.dma_start(out=out_view, in_=out_all_3d)

    return nc


def run(x, weight, bias, feature_importance, k, trace=False, trace_kwargs=None):
    from concourse.bass_utils import run_bass_kernel_spmd

    x = np.ascontiguousarray(np.asarray(x, dtype=np.float32))
    weight = np.asarray(weight, dtype=np.float32)
    bias = np.asarray(bias, dtype=np.float32)
    fi = np.asarray(feature_importance, dtype=np.float32)
    k = int(k)

    # top-k by |fi|, ties broken by lower index (matches jax.lax.top_k)
    order = np.lexsort((np.arange(fi.shape[0]), -np.abs(fi)))
    idx = np.sort(order[:k])
    pairs = _make_pairs(idx)

    wT_sel = np.ascontiguousarray(weight[:, idx].T)          # [k, 8]
    bias_tile = np.ascontiguousarray(
        np.broadcast_to(bias[None, :], (P, OUT_F)).astype(np.float32)
    )
    ident = np.eye(P, dtype=np.float32)

    nc = build_nc(pairs, k)
    if not nc.is_finalized():
        nc.finalize()

    in_maps = []
    for c in range(NCORES):
        in_maps.append({
            "x": x[c * ROWS:(c + 1) * ROWS],
            "wt": wT_sel,
            "biasb": bias_tile,
            "ident": ident,
        })

    kw = {}
    if trace:
        kw["trace"] = True
        if trace_kwargs:
            kw.update(trace_kwargs)
    try:
        res = run_bass_kernel_spmd(nc, in_maps, list(range(NCORES)), **kw)
    except ModuleNotFoundError:
        if not trace:
            raise
        # NTFF profile hook unavailable in this environment; rerun untraced.
        res = run_bass_kernel_spmd(nc, in_maps, list(range(NCORES)))
    out = np.concatenate([res.results[c]["out"] for c in range(NCORES)], axis=0)
    return out, res.exec_time_ns


def kernel(x, weight, bias, feature_importance, k):
    out, _ = run(x, weight, bias, feature_importance, k, trace=False)
    return out



# revision 5
# speedup vs baseline: 6.0942x; 6.0942x over previous
"""KSparseLinear forward on 8 Trainium2 NeuronCores (column-gather).

out = (x * mask) @ weight.T + bias, where mask keeps the top-k=64
|feature_importance| columns of the 4096 input features.

Only the k selected columns of x matter, so instead of streaming the
full 1 GiB of x (64 full-row tile DMAs per core), each core gathers
just its shard's k columns with strided column DMAs — one DMA per
column run, each a [128-partition, 64-tile] access pattern — spread
across the SP/ACT/Pool DMA queues.  Per 128-row tile the PE transposes
the gathered [128, 64] block (via identity), ACT copies it to SBUF and
one small matmul produces [128, 8]; DVE adds the bias.  The output is
written back with one batched DMA.
"""

import numpy as np

N_FULL, IN_F, OUT_F = 65536, 4096, 8
NCORES = 8
ROWS = N_FULL // NCORES      # 8192 rows per core
P = 128                      # partitions / rows per tile
NT = ROWS // P               # 64 row-tiles per core


def build_nc(idx_cols, k):
    import concourse.bass as bass
    import concourse.mybir as mybir
    from concourse import tile
    from concourse.bacc import Bacc

    f32 = mybir.dt.float32
    nc = Bacc()

    x_d = nc.declare_dram_parameter("x", [ROWS, IN_F], f32, isOutput=False)
    wt_d = nc.declare_dram_parameter("wt", [k, OUT_F], f32, isOutput=False)
    bias_d = nc.declare_dram_parameter("biasb", [P, OUT_F], f32, isOutput=False)
    id_d = nc.declare_dram_parameter("ident", [P, P], f32, isOutput=False)
    out_d = nc.declare_dram_parameter("out", [ROWS, OUT_F], f32, isOutput=True)

    QN = ("sync", "scalar", "gpsimd")

    with tile.TileContext(nc) as tc:
        with (
            tc.tile_pool(name="consts", bufs=1) as consts,
            tc.tile_pool(name="xp", bufs=1) as xp,
            tc.tile_pool(name="xt", bufs=3) as xtp,
            tc.tile_pool(name="psT", bufs=2, space=bass.MemorySpace.PSUM) as psT,
            tc.tile_pool(name="psO", bufs=2, space=bass.MemorySpace.PSUM) as psO,
            tc.tile_pool(name="outp", bufs=1) as outp,
        ):
            wt_sb = consts.tile([k, OUT_F], f32)
            nc.sync.dma_start(out=wt_sb[:], in_=wt_d[:, :])
            bias_sb = consts.tile([P, OUT_F], f32)
            nc.scalar.dma_start(out=bias_sb[:], in_=bias_d[:, :])
            id_sb = consts.tile([P, P], f32)
            nc.gpsimd.dma_start(out=id_sb[:], in_=id_d[:, :])

            # gather the k needed columns: X[p, t*k + j] = x[t*128+p, c_j]
            X = xp.tile([P, NT * k], f32)
            X3 = X[:].rearrange("p (t j) -> p t j", j=k)
            qload = {"sync": 500.0, "scalar": 500.0, "gpsimd": 500.0}
            with nc.allow_non_contiguous_dma(reason="host col gather"):
                jj = 0
                while jj < k:
                    ln = 1
                    while (jj + ln < k
                           and idx_cols[jj + ln] == idx_cols[jj] + ln):
                        ln += 1
                    q = min(QN, key=lambda n: qload[n])
                    qload[q] += max(500.0, NT * ln * 4 * 0.3855 * 2)
                    c0 = int(idx_cols[jj])
                    in_ap = x_d[:, c0:c0 + ln].rearrange(
                        "(t p) j -> p t j", p=P
                    )
                    getattr(nc, q).dma_start(
                        out=X3[:, :, jj:jj + ln], in_=in_ap
                    )
                    jj += ln

            out_all = outp.tile([P, NT * OUT_F], f32)

            for t in range(NT):
                xT_ps = psT.tile([k, P], f32)
                nc.tensor.transpose(xT_ps[:], X[:, t * k:(t + 1) * k], id_sb[:])
                xT_sb = xtp.tile([k, P], f32)
                nc.scalar.copy(xT_sb[:], xT_ps[:])

                o_ps = psO.tile([P, OUT_F], f32)
                nc.tensor.matmul(o_ps[:], xT_sb[:], wt_sb[:],
                                 start=True, stop=True)
                nc.vector.tensor_add(
                    out_all[:, t * OUT_F:(t + 1) * OUT_F], o_ps[:], bias_sb[:]
                )

            out_view = out_d[:, :].rearrange("(t p) o -> p t o", p=P)
            out_all_3d = out_all[:].rearrange("p (t o) -> p t o", o=OUT_F)
            nc.sync.dma_start(out=out_view, in_=out_all_3d)

    return nc


def host_consts(idx, weight, bias):
    return {
        "wt": np.ascontiguousarray(weight[:, idx].T),
        "biasb": np.ascontiguousarray(
            np.broadcast_to(bias[None, :], (P, OUT_F)).astype(np.float32)
        ),
        "ident": np.eye(P, dtype=np.float32),
    }


def topk_indices(feature_importance, k):
    """Top-k by |fi|, ties broken by lower index (matches jax.lax.top_k)."""
    fi = np.asarray(feature_importance, dtype=np.float32)
    order = np.lexsort((np.arange(fi.shape[0]), -np.abs(fi)))
    return np.sort(order[:k])


def run(x, weight, bias, feature_importance, k, trace=False, trace_kwargs=None):
    from concourse.bass_utils import run_bass_kernel_spmd

    x = np.ascontiguousarray(np.asarray(x, dtype=np.float32))
    weight = np.asarray(weight, dtype=np.float32)
    bias = np.asarray(bias, dtype=np.float32)
    k = int(k)

    idx = topk_indices(feature_importance, k)
    nc = build_nc(idx, k)
    if not nc.is_finalized():
        nc.finalize()

    consts = host_consts(idx, weight, bias)
    in_maps = []
    for c in range(NCORES):
        m = {"x": x[c * ROWS:(c + 1) * ROWS]}
        m.update(consts)
        in_maps.append(m)

    kw = {}
    if trace:
        kw["trace"] = True
        if trace_kwargs:
            kw.update(trace_kwargs)
    try:
        res = run_bass_kernel_spmd(nc, in_maps, list(range(NCORES)), **kw)
    except ModuleNotFoundError:
        if not trace:
            raise
        res = run_bass_kernel_spmd(nc, in_maps, list(range(NCORES)))
    out = np.concatenate([res.results[c]["out"] for c in range(NCORES)], axis=0)
    return out, res.exec_time_ns


def kernel(x, weight, bias, feature_importance, k):
    out, _ = run(x, weight, bias, feature_importance, k, trace=False)
    return out


# revision 6
# speedup vs baseline: 6.8168x; 1.1186x over previous
"""KSparseLinear forward on 8 Trainium2 NeuronCores (column-gather).

out = (x * mask) @ weight.T + bias, where mask keeps the top-k=64
|feature_importance| columns of the 4096 input features.

Only the k selected columns of x matter, so instead of streaming the
full 1 GiB of x (64 full-row tile DMAs per core), each core gathers
just its shard's k columns with strided column DMAs — one DMA per
column run, each a [128-partition, 64-tile] access pattern — spread
across the SP/ACT/Pool DMA queues.  Per 128-row tile the PE transposes
the gathered [128, 64] block (via identity), ACT copies it to SBUF and
one small matmul produces [128, 8]; DVE adds the bias.  The output is
written back with one batched DMA.
"""

import numpy as np

N_FULL, IN_F, OUT_F = 65536, 4096, 8
NCORES = 8
ROWS = N_FULL // NCORES      # 8192 rows per core
P = 128                      # partitions / rows per tile
NT = ROWS // P               # 64 row-tiles per core


def build_nc(idx_cols, k):
    import concourse.bass as bass
    import concourse.mybir as mybir
    from concourse import tile
    from concourse.bacc import Bacc

    f32 = mybir.dt.float32
    nc = Bacc()

    x_d = nc.declare_dram_parameter("x", [ROWS, IN_F], f32, isOutput=False)
    wt_d = nc.declare_dram_parameter("wt", [P, OUT_F], f32, isOutput=False)
    bias_d = nc.declare_dram_parameter("biasb", [P, OUT_F], f32, isOutput=False)
    id_d = nc.declare_dram_parameter("ident", [P, P], f32, isOutput=False)
    out_d = nc.declare_dram_parameter("out", [ROWS, OUT_F], f32, isOutput=True)

    QN = ("sync", "scalar", "gpsimd")

    with tile.TileContext(nc) as tc:
        with (
            tc.tile_pool(name="consts", bufs=1) as consts,
            tc.tile_pool(name="xp", bufs=1) as xp,
            tc.tile_pool(name="xt", bufs=3) as xtp,
            tc.tile_pool(name="psT", bufs=2, space=bass.MemorySpace.PSUM) as psT,
            tc.tile_pool(name="psO", bufs=2, space=bass.MemorySpace.PSUM) as psO,
            tc.tile_pool(name="outp", bufs=1) as outp,
        ):
            wt_sb = consts.tile([P, OUT_F], f32)
            nc.sync.dma_start(out=wt_sb[:], in_=wt_d[:, :])
            bias_sb = consts.tile([P, OUT_F], f32)
            nc.scalar.dma_start(out=bias_sb[:], in_=bias_d[:, :])
            id_sb = consts.tile([P, P], f32)
            nc.gpsimd.dma_start(out=id_sb[:], in_=id_d[:, :])

            # gather the k needed columns: X[p, t*k + j] = x[t*128+p, c_j]
            X = xp.tile([P, NT * k], f32)
            X3 = X[:].rearrange("p (t j) -> p t j", j=k)
            qload = {"sync": 500.0, "scalar": 500.0, "gpsimd": 500.0}
            with nc.allow_non_contiguous_dma(reason="host col gather"):
                jj = 0
                while jj < k:
                    ln = 1
                    while (jj + ln < k
                           and idx_cols[jj + ln] == idx_cols[jj] + ln):
                        ln += 1
                    q = min(QN, key=lambda n: qload[n])
                    qload[q] += max(500.0, NT * ln * 4 * 0.3855 * 2)
                    c0 = int(idx_cols[jj])
                    in_ap = x_d[:, c0:c0 + ln].rearrange(
                        "(t p) j -> p t j", p=P
                    )
                    getattr(nc, q).dma_start(
                        out=X3[:, :, jj:jj + ln], in_=in_ap
                    )
                    jj += ln

            out_all = outp.tile([P, NT * OUT_F], f32)

            TBT = P // k if P % k == 0 else 1   # tiles per transpose batch
            for b in range(NT // TBT):
                xT_ps = psT.tile([TBT * k, P], f32)
                nc.tensor.transpose(
                    xT_ps[:], X[:, b * TBT * k:(b + 1) * TBT * k], id_sb[:]
                )
                xT_sb = xtp.tile([TBT * k, P], f32)
                nc.vector.tensor_copy(xT_sb[:], xT_ps[:])
                for u in range(TBT):
                    t = b * TBT + u
                    o_ps = psO.tile([P, OUT_F], f32)
                    nc.tensor.matmul(
                        o_ps[:], xT_sb[u * k:(u + 1) * k, :],
                        wt_sb[u * k:(u + 1) * k, :], start=True, stop=True
                    )
                    nc.vector.tensor_add(
                        out_all[:, t * OUT_F:(t + 1) * OUT_F], o_ps[:], bias_sb[:]
                    )

            out_view = out_d[:, :].rearrange("(t p) o -> p t o", p=P)
            out_all_3d = out_all[:].rearrange("p (t o) -> p t o", o=OUT_F)
            nc.sync.dma_start(out=out_view, in_=out_all_3d)

    return nc


def host_consts(idx, weight, bias):
    return {
        "wt": np.ascontiguousarray(
            np.tile(weight[:, idx].T, (max(1, P // len(idx)), 1))
        ),
        "biasb": np.ascontiguousarray(
            np.broadcast_to(bias[None, :], (P, OUT_F)).astype(np.float32)
        ),
        "ident": np.eye(P, dtype=np.float32),
    }


def topk_indices(feature_importance, k):
    """Top-k by |fi|, ties broken by lower index (matches jax.lax.top_k)."""
    fi = np.asarray(feature_importance, dtype=np.float32)
    order = np.lexsort((np.arange(fi.shape[0]), -np.abs(fi)))
    return np.sort(order[:k])


def run(x, weight, bias, feature_importance, k, trace=False, trace_kwargs=None):
    from concourse.bass_utils import run_bass_kernel_spmd

    x = np.ascontiguousarray(np.asarray(x, dtype=np.float32))
    weight = np.asarray(weight, dtype=np.float32)
    bias = np.asarray(bias, dtype=np.float32)
    k = int(k)

    idx = topk_indices(feature_importance, k)
    nc = build_nc(idx, k)
    if not nc.is_finalized():
        nc.finalize()

    consts = host_consts(idx, weight, bias)
    in_maps = []
    for c in range(NCORES):
        m = {"x": x[c * ROWS:(c + 1) * ROWS]}
        m.update(consts)
        in_maps.append(m)

    kw = {}
    if trace:
        kw["trace"] = True
        if trace_kwargs:
            kw.update(trace_kwargs)
    try:
        res = run_bass_kernel_spmd(nc, in_maps, list(range(NCORES)), **kw)
    except ModuleNotFoundError:
        if not trace:
            raise
        res = run_bass_kernel_spmd(nc, in_maps, list(range(NCORES)))
    out = np.concatenate([res.results[c]["out"] for c in range(NCORES)], axis=0)
    return out, res.exec_time_ns


def kernel(x, weight, bias, feature_importance, k):
    out, _ = run(x, weight, bias, feature_importance, k, trace=False)
    return out


# revision 8
# speedup vs baseline: 7.2905x; 1.0695x over previous
"""KSparseLinear forward on 8 Trainium2 NeuronCores (column-gather).

out = (x * mask) @ weight.T + bias, where mask keeps the top-k=64
|feature_importance| columns of the 4096 input features.

Only the k selected columns of x matter, so instead of streaming the
full 1 GiB of x (64 full-row tile DMAs per core), each core gathers
just its shard's k columns with strided column DMAs — one DMA per
column run, each a [128-partition, 64-tile] access pattern — spread
across the SP/ACT/Pool DMA queues.  The PE transposes gathered
[128, 128] blocks (two row-tiles at a time, via identity), DVE copies
them to SBUF, and one small matmul per row-tile produces [128, 8];
DVE adds the bias.  The output is written back with one batched DMA.
"""

import numpy as np

N_FULL, IN_F, OUT_F = 65536, 4096, 8
NCORES = 8
ROWS = N_FULL // NCORES      # 8192 rows per core
P = 128                      # partitions / rows per tile
NT = ROWS // P               # 64 row-tiles per core


def build_nc(idx_cols, k):
    import concourse.bass as bass
    import concourse.mybir as mybir
    from concourse import tile
    from concourse.bacc import Bacc

    f32 = mybir.dt.float32
    nc = Bacc()

    x_d = nc.declare_dram_parameter("x", [ROWS, IN_F], f32, isOutput=False)
    wt_d = nc.declare_dram_parameter("wt", [P, OUT_F], f32, isOutput=False)
    bias_d = nc.declare_dram_parameter("biasb", [P, OUT_F], f32, isOutput=False)
    id_d = nc.declare_dram_parameter("ident", [P, P], f32, isOutput=False)
    out_d = nc.declare_dram_parameter("out", [ROWS, OUT_F], f32, isOutput=True)

    QN = ("sync", "scalar", "gpsimd")

    with tile.TileContext(nc) as tc:
        with (
            tc.tile_pool(name="consts", bufs=1) as consts,
            tc.tile_pool(name="xp", bufs=1) as xp,
            tc.tile_pool(name="xt", bufs=3) as xtp,
            tc.tile_pool(name="psT", bufs=2, space=bass.MemorySpace.PSUM) as psT,
            tc.tile_pool(name="psO", bufs=2, space=bass.MemorySpace.PSUM) as psO,
            tc.tile_pool(name="outp", bufs=1) as outp,
        ):
            wt_sb = consts.tile([P, OUT_F], f32)
            nc.sync.dma_start(out=wt_sb[:], in_=wt_d[:, :])
            bias_sb = consts.tile([P, OUT_F], f32)
            nc.scalar.dma_start(out=bias_sb[:], in_=bias_d[:, :])
            id_sb = consts.tile([P, P], f32)
            nc.gpsimd.dma_start(out=id_sb[:], in_=id_d[:, :])

            # gather the k needed columns: X[p, t*k + j] = x[t*128+p, c_j]
            X = xp.tile([P, NT * k], f32)
            X3 = X[:].rearrange("p (t j) -> p t j", j=k)
            qload = {"sync": 500.0, "scalar": 3500.0, "gpsimd": 500.0}
            with nc.allow_non_contiguous_dma(reason="host col gather"):
                jj = 0
                while jj < k:
                    ln = 1
                    while (jj + ln < k
                           and idx_cols[jj + ln] == idx_cols[jj] + ln):
                        ln += 1
                    q = min(QN, key=lambda n: qload[n])
                    qload[q] += max(500.0, NT * ln * 4 * 0.3855 * 2)
                    c0 = int(idx_cols[jj])
                    in_ap = x_d[:, c0:c0 + ln].rearrange(
                        "(t p) j -> p t j", p=P
                    )
                    getattr(nc, q).dma_start(
                        out=X3[:, :, jj:jj + ln], in_=in_ap
                    )
                    jj += ln

            QNT = NT // 4                      # row-tiles per output quarter
            out_q = [outp.tile([P, QNT * OUT_F], f32, name=f"out_q{q}")
                     for q in range(4)]
            out_view = out_d[:, :].rearrange("(t p) o -> p t o", p=P)

            TBT = P // k if P % k == 0 else 1   # tiles per transpose batch
            for b in range(NT // TBT):
                xT_ps = psT.tile([TBT * k, P], f32)
                nc.tensor.transpose(
                    xT_ps[:], X[:, b * TBT * k:(b + 1) * TBT * k], id_sb[:]
                )
                xT_sb = xtp.tile([TBT * k, P], f32)
                if b % 2 == 0:
                    nc.vector.tensor_copy(xT_sb[:], xT_ps[:])
                else:
                    nc.scalar.copy(xT_sb[:], xT_ps[:])
                for u in range(TBT):
                    t = b * TBT + u
                    q, tq = t // QNT, t % QNT
                    o_ps = psO.tile([P, OUT_F], f32)
                    nc.tensor.matmul(
                        o_ps[:], xT_sb[u * k:(u + 1) * k, :],
                        wt_sb[u * k:(u + 1) * k, :], start=True, stop=True
                    )
                    nc.vector.tensor_add(
                        out_q[q][:, tq * OUT_F:(tq + 1) * OUT_F],
                        o_ps[:], bias_sb[:]
                    )
                    if tq == QNT - 1:
                        eng = nc.scalar if q % 2 == 0 else nc.sync
                        eng.dma_start(
                            out=out_view[:, q * QNT:(q + 1) * QNT],
                            in_=out_q[q][:].rearrange(
                                "p (t o) -> p t o", o=OUT_F
                            ),
                        )

    return nc


def host_consts(idx, weight, bias):
    return {
        "wt": np.ascontiguousarray(
            np.tile(weight[:, idx].T, (max(1, P // len(idx)), 1))
        ),
        "biasb": np.ascontiguousarray(
            np.broadcast_to(bias[None, :], (P, OUT_F)).astype(np.float32)
        ),
        "ident": np.eye(P, dtype=np.float32),
    }


def topk_indices(feature_importance, k):
    """Top-k by |fi|, ties broken by lower index (matches jax.lax.top_k)."""
    fi = np.asarray(feature_importance, dtype=np.float32)
    order = np.lexsort((np.arange(fi.shape[0]), -np.abs(fi)))
    return np.sort(order[:k])


def run(x, weight, bias, feature_importance, k, trace=False, trace_kwargs=None):
    from concourse.bass_utils import run_bass_kernel_spmd

    x = np.ascontiguousarray(np.asarray(x, dtype=np.float32))
    weight = np.asarray(weight, dtype=np.float32)
    bias = np.asarray(bias, dtype=np.float32)
    k = int(k)

    idx = topk_indices(feature_importance, k)
    nc = build_nc(idx, k)
    if not nc.is_finalized():
        nc.finalize()

    consts = host_consts(idx, weight, bias)
    in_maps = []
    for c in range(NCORES):
        m = {"x": x[c * ROWS:(c + 1) * ROWS]}
        m.update(consts)
        in_maps.append(m)

    kw = {}
    if trace:
        kw["trace"] = True
        if trace_kwargs:
            kw.update(trace_kwargs)
    try:
        res = run_bass_kernel_spmd(nc, in_maps, list(range(NCORES)), **kw)
    except ModuleNotFoundError:
        if not trace:
            raise
        res = run_bass_kernel_spmd(nc, in_maps, list(range(NCORES)))
    out = np.concatenate([res.results[c]["out"] for c in range(NCORES)], axis=0)
    return out, res.exec_time_ns


def kernel(x, weight, bias, feature_importance, k):
    out, _ = run(x, weight, bias, feature_importance, k, trace=False)
    return out


# revision 16
# speedup vs baseline: 7.6529x; 1.0497x over previous
"""KSparseLinear forward on 8 Trainium2 NeuronCores (column-gather).

out = (x * mask) @ weight.T + bias, where mask keeps the top-k=64
|feature_importance| columns of the 4096 input features.

Only the k selected columns of x matter, so instead of streaming the
full 1 GiB of x (64 full-row tile DMAs per core), each core gathers
just its shard's k columns with strided column DMAs — one DMA per
column run, each a [128-partition, 64-tile] access pattern — spread
across the SP/ACT/Pool DMA queues.  The PE transposes gathered
[128, 128] blocks (two row-tiles at a time, via identity), DVE copies
them to SBUF, and one small matmul per row-tile produces [128, 8];
DVE adds the bias.  The output is written back with one batched DMA.
"""

import numpy as np

N_FULL, IN_F, OUT_F = 65536, 4096, 8
NCORES = 8
ROWS = N_FULL // NCORES      # 8192 rows per core
P = 128                      # partitions / rows per tile
NT = ROWS // P               # 64 row-tiles per core


def build_nc(idx_cols, k):
    import concourse.bass as bass
    import concourse.mybir as mybir
    from concourse import tile
    from concourse.bacc import Bacc

    f32 = mybir.dt.float32
    nc = Bacc()

    x_d = nc.declare_dram_parameter("x", [ROWS, IN_F], f32, isOutput=False)
    wt_d = nc.declare_dram_parameter("wt", [P, OUT_F], f32, isOutput=False)
    bias_d = nc.declare_dram_parameter("biasb", [P, OUT_F], f32, isOutput=False)
    id_d = nc.declare_dram_parameter("ident", [P, P], f32, isOutput=False)
    out_d = nc.declare_dram_parameter("out", [ROWS, OUT_F], f32, isOutput=True)

    QN = ("sync", "scalar", "gpsimd")

    with tile.TileContext(nc) as tc:
        with (
            tc.tile_pool(name="consts", bufs=1) as consts,
            tc.tile_pool(name="xp", bufs=1) as xp,
            tc.tile_pool(name="xt", bufs=4) as xtp,
            tc.tile_pool(name="psT", bufs=3, space=bass.MemorySpace.PSUM) as psT,
            tc.tile_pool(name="psO", bufs=4, space=bass.MemorySpace.PSUM) as psO,
            tc.tile_pool(name="outp", bufs=1) as outp,
        ):
            wt_sb = consts.tile([P, OUT_F], f32)
            nc.sync.dma_start(out=wt_sb[:], in_=wt_d[:, :])
            bias_sb = consts.tile([P, OUT_F], f32)
            nc.scalar.dma_start(out=bias_sb[:], in_=bias_d[:, :])
            id_sb = consts.tile([P, P], f32)
            nc.gpsimd.dma_start(out=id_sb[:], in_=id_d[:, :])

            # gather the k needed columns: X[p, t*k + j] = x[t*128+p, c_j]
            X = xp.tile([P, NT * k], f32)
            X3 = X[:].rearrange("p (t j) -> p t j", j=k)
            qload = {"sync": 500.0, "scalar": 3500.0, "gpsimd": 500.0}
            with nc.allow_non_contiguous_dma(reason="host col gather"):
                jj = 0
                while jj < k:
                    ln = 1
                    while (jj + ln < k
                           and idx_cols[jj + ln] == idx_cols[jj] + ln):
                        ln += 1
                    q = min(QN, key=lambda n: qload[n])
                    qload[q] += max(500.0, NT * ln * 4 * 0.3855 * 2)
                    c0 = int(idx_cols[jj])
                    in_ap = x_d[:, c0:c0 + ln].rearrange(
                        "(t p) j -> p t j", p=P
                    )
                    getattr(nc, q).dma_start(
                        out=X3[:, :, jj:jj + ln], in_=in_ap
                    )
                    jj += ln

            QNT = NT // 4                      # row-tiles per output quarter
            out_q = [outp.tile([P, QNT * OUT_F], f32, name=f"out_q{q}")
                     for q in range(4)]
            out_view = out_d[:, :].rearrange("(t p) o -> p t o", p=P)

            TBT = P // k if P % k == 0 else 1   # tiles per transpose batch
            for b in range(NT // TBT):
                xT_ps = psT.tile([TBT * k, P], f32)
                nc.tensor.transpose(
                    xT_ps[:], X[:, b * TBT * k:(b + 1) * TBT * k], id_sb[:]
                )
                xT_sb = xtp.tile([TBT * k, P], f32)
                if b % 2 == 0:
                    nc.vector.tensor_copy(xT_sb[:], xT_ps[:])
                else:
                    nc.scalar.copy(xT_sb[:], xT_ps[:])
                for u in range(TBT):
                    t = b * TBT + u
                    q, tq = t // QNT, t % QNT
                    o_ps = psO.tile([P, OUT_F], f32)
                    nc.tensor.matmul(
                        o_ps[:], xT_sb[u * k:(u + 1) * k, :],
                        wt_sb[u * k:(u + 1) * k, :], start=True, stop=True
                    )
                    nc.vector.tensor_add(
                        out_q[q][:, tq * OUT_F:(tq + 1) * OUT_F],
                        o_ps[:], bias_sb[:]
                    )
                    if tq == QNT - 1:
                        eng = nc.scalar if q % 2 == 0 else nc.sync
                        eng.dma_start(
                            out=out_view[:, q * QNT:(q + 1) * QNT],
                            in_=out_q[q][:].rearrange(
                                "p (t o) -> p t o", o=OUT_F
                            ),
                        )

    return nc


def host_consts(idx, weight, bias):
    return {
        "wt": np.ascontiguousarray(
            np.tile(weight[:, idx].T, (max(1, P // len(idx)), 1))
        ),
        "biasb": np.ascontiguousarray(
            np.broadcast_to(bias[None, :], (P, OUT_F)).astype(np.float32)
        ),
        "ident": np.eye(P, dtype=np.float32),
    }


def topk_indices(feature_importance, k):
    """Top-k by |fi|, ties broken by lower index (matches jax.lax.top_k)."""
    fi = np.asarray(feature_importance, dtype=np.float32)
    order = np.lexsort((np.arange(fi.shape[0]), -np.abs(fi)))
    return np.sort(order[:k])


def run(x, weight, bias, feature_importance, k, trace=False, trace_kwargs=None):
    from concourse.bass_utils import run_bass_kernel_spmd

    x = np.ascontiguousarray(np.asarray(x, dtype=np.float32))
    weight = np.asarray(weight, dtype=np.float32)
    bias = np.asarray(bias, dtype=np.float32)
    k = int(k)

    idx = topk_indices(feature_importance, k)
    nc = build_nc(idx, k)
    if not nc.is_finalized():
        nc.finalize()

    consts = host_consts(idx, weight, bias)
    in_maps = []
    for c in range(NCORES):
        m = {"x": x[c * ROWS:(c + 1) * ROWS]}
        m.update(consts)
        in_maps.append(m)

    kw = {}
    if trace:
        kw["trace"] = True
        if trace_kwargs:
            kw.update(trace_kwargs)
    try:
        res = run_bass_kernel_spmd(nc, in_maps, list(range(NCORES)), **kw)
    except ModuleNotFoundError:
        if not trace:
            raise
        res = run_bass_kernel_spmd(nc, in_maps, list(range(NCORES)))
    out = np.concatenate([res.results[c]["out"] for c in range(NCORES)], axis=0)
    return out, res.exec_time_ns


def kernel(x, weight, bias, feature_importance, k):
    out, _ = run(x, weight, bias, feature_importance, k, trace=False)
    return out
